# revision 10
# baseline (speedup 1.0000x reference)
"""BigBird block-sparse attention on 8 Trainium2 NeuronCores.

Sharding: core = (batch b, head-group hg): b = core//4, hg = core%4.
Each core computes, for its batch and its 4 heads (all f16 inputs):
  qT/kT = (W{q,k}[hs] @ x.T)            [256, 2048]  (q pre-scaled by 1/8)
  v     = x @ Wv[hs].T                  [2048, 256]  in score-pair row layout
  Key blocks processed in PAIRS (0,31),(1,2),(3,4)..(29,30): one matmul
  computes transposed scores for both blocks of a pair (128 psum rows):
  S.T   = [k_a|k_b] @ q_span.T          write [128, span] per piece
  expS  = exp(S.T) packed in PSUM fills, evicted to SBUF (fp16),
          per-half holes zeroed (block kept by only one of a/b)
  outT  = [v_a|1 ; v_b|1].T @ expS      [65, 2048] accumulated in PSUM
  attnT = outT[0:64] * (1/outT[64]) per head  -> [256, 2048] f16
  out  += attnT.T @ Wo[:, hs].T         [2048, 1024] partial sum over heads
Host gathers: out[b] = sum over the 4 head-group cores of that batch.
"""

import os
import sys

import numpy as np

_B, _L, _D = 2, 2048, 1024
_H, _HD, _BLK = 16, 64, 64
_NB = _L // _BLK  # 32
_NCORES = 8
_HPC = 4  # heads per core
_FILLW = 1024  # packed-psum fill width (2 PSUM banks, f32)

_cache = {}


# --------------------------------------------------------------------------
# host-side plan: derive the block mask structure once
# --------------------------------------------------------------------------
def _build_plan(bm):
    """bm: [NB, NB] bool block mask (bm[i, j] = q-block i attends key-block j).

    Key blocks are processed in pairs: pair 0 = (0, NB-1), pair t>=1 =
    (2t-1, 2t).  Pair t's scores live on psum rows 0:64 (first block) and
    64:128 (second block) of shared q columns (the union of both blocks'
    kept q-blocks).

    Returns dict with:
      fills: list of fills; each is a list of pieces
             dict(t, q0, n, off, g2, avs, holes) where
             avs = [(a, nn, off2)] AV sub-pieces split at outT psum banks
             holes = [(half, q)] blocks to zero in expS after exp
      pair_blocks: [(a, b)] per pair t
      n_fills, av_flags
    """
    NB = bm.shape[0]
    GAP = int(os.environ.get("BIGBIRD_GAP", "0"))
    pairs = [(0, NB - 1)] + [(2 * t - 1, 2 * t) for t in range(1, NB // 2)]
    Q = [set(np.nonzero(bm[:, j])[0].tolist()) for j in range(NB)]

    fills = [[]]
    cur = [0]

    def close_fill():
        if fills[-1]:
            fills.append([])
        cur[0] = 0

    for t, (a, b) in enumerate(pairs):
        Qa, Qb = Q[a], Q[b]
        U = sorted(Qa | Qb)
        # merge q blocks into runs, allowing holes up to GAP blocks
        runs = []
        s = p = U[0]
        for x in U[1:]:
            if x - p <= GAP + 1:
                p = x
            else:
                runs.append((s, p))
                s = p = x
        runs.append((s, p))
        # fuse isolated {0} and {NB-1} singles into one strided matmul
        g2 = (len(runs) >= 2 and runs[0] == (0, 0)
              and runs[-1] == (NB - 1, NB - 1))
        if g2:
            runs = runs[1:-1]

        def emit(q0, n, is_g2):
            off = cur[0]
            cur[0] += (128 if is_g2 else n * _BLK)
            if is_g2:
                span = [0, NB - 1]
                avs = [(0, 1, off), (NB - 1, 1, off + 64)]
            else:
                span = list(range(q0, q0 + n))
                avs = []
                x = q0
                while x < q0 + n:
                    lim = min(q0 + n, ((x // 8) + 1) * 8)
                    avs.append((x, lim - x, off + (x - q0) * _BLK))
                    x = lim
            holes = ([(0, q) for q in span if q not in Qa]
                     + [(1, q) for q in span if q not in Qb])
            fills[-1].append(dict(t=t, q0=q0, n=n, off=off, g2=is_g2,
                                  avs=avs, holes=holes))

        if g2:
            # 128 cols; must not cross a 512-col psum bank boundary
            if cur[0] % 512 > 512 - 128:
                cur[0] = (cur[0] // 512 + 1) * 512
                if cur[0] >= _FILLW:
                    close_fill()
            emit(0, 2, True)
        for (s, e) in runs:
            q = s
            n_left = e - s + 1
            while n_left > 0:
                if cur[0] >= _FILLW:
                    close_fill()
                room = (512 - cur[0] % 512) // _BLK
                take = min(n_left, 8, room)
                emit(q, take, False)
                q += take
                n_left -= take
    if not fills[-1]:
        fills.pop()

    # outT psum-bank start/stop flags: first/last AV piece touching each
    # 512-col (8 q-block) bank, in emission order
    exec_order = []
    for fi, fill in enumerate(fills):
        for pi, pc in enumerate(fill):
            for ai, (x, nn, off2) in enumerate(pc["avs"]):
                exec_order.append(((fi, pi, ai), x // 8))
    first_seen = set()
    last_key = {}
    for key, bank in exec_order:
        last_key[bank] = key
    av_flags = {}
    for key, bank in exec_order:
        av_flags[key] = (bank not in first_seen, last_key[bank] == key)
        first_seen.add(bank)

    return dict(fills=fills, n_fills=len(fills), av_flags=av_flags,
                pair_blocks=pairs)


# --------------------------------------------------------------------------
# numpy simulator of the planned pipeline (used by test_plan.py)
# --------------------------------------------------------------------------
def _sim_plan(plan, q, k, v):
    """q, k, v: [L, 64] f32 (q pre-scaled by 1/8). Returns attn out [L, 64]."""
    nf = plan["n_fills"]
    pairs = plan["pair_blocks"]
    expS = np.zeros((128, nf * _FILLW), np.float32)
    outT = np.zeros((65, _L), np.float64)
    for fi, fill in enumerate(plan["fills"]):
        ps = np.zeros((128, _FILLW), np.float32)
        for pc in fill:
            t, q0, n, off = pc["t"], pc["q0"], pc["n"], pc["off"]
            a, b = pairs[t]
            if pc["g2"]:
                qsel = np.concatenate([q[0:64], q[(_NB - 1) * 64:]], axis=0)
            else:
                qsel = q[q0 * 64:(q0 + n) * 64]
            kp = np.concatenate([k[a * 64:(a + 1) * 64],
                                 k[b * 64:(b + 1) * 64]], axis=0)
            ps[:, off:off + qsel.shape[0]] = kp @ qsel.T
        expS[:, fi * _FILLW:(fi + 1) * _FILLW] = np.exp(ps)
        for pc in fill:
            span = ([0, _NB - 1] if pc["g2"]
                    else list(range(pc["q0"], pc["q0"] + pc["n"])))
            for (half, qb) in pc["holes"]:
                ci = span.index(qb)
                expS[half * 64:half * 64 + 64,
                     fi * _FILLW + pc["off"] + ci * 64:
                     fi * _FILLW + pc["off"] + (ci + 1) * 64] = 0.0
        for pc in fill:
            t = pc["t"]
            a, b = pairs[t]
            vp = np.concatenate([
                np.concatenate([v[a * 64:(a + 1) * 64],
                                np.ones((64, 1), np.float32)], axis=1),
                np.concatenate([v[b * 64:(b + 1) * 64],
                                np.ones((64, 1), np.float32)], axis=1),
            ], axis=0)  # [128, 65]
            for (x, nn, off2) in pc["avs"]:
                e = expS[:, fi * _FILLW + off2: fi * _FILLW + off2 + nn * 64]
                outT[:, x * 64:(x + nn) * 64] += vp.T @ e
    return (outT[0:64] / outT[64]).T


# --------------------------------------------------------------------------
# bass kernel build
# --------------------------------------------------------------------------
def _build_nc(plan):
    import concourse.bacc as bacc
    import concourse.mybir as mybir
    from concourse.tile import TileContext

    f32r = mybir.dt.float32r
    f32 = mybir.dt.float32
    f16 = mybir.dt.float16
    EXP = mybir.ActivationFunctionType.Exp
    COPY = mybir.ActivationFunctionType.Copy

    NKC = _D // 128   # 8 contraction chunks
    NM = _L // 128    # 16 L tiles
    nf = plan["n_fills"]
    pairs = plan["pair_blocks"]

    nc = bacc.Bacc(None, target_bir_lowering=False)

    xt = nc.dram_tensor("xt", [_D, _L], f16, kind="ExternalInput")
    wq = nc.dram_tensor("wq", [_D, 256], f16, kind="ExternalInput")
    wk = nc.dram_tensor("wk", [_D, 256], f16, kind="ExternalInput")
    wv = nc.dram_tensor("wv", [_D, 256], f16, kind="ExternalInput")
    wo = nc.dram_tensor("wo", [256, _D], f16, kind="ExternalInput")
    out = nc.dram_tensor("out", [_L, _D], f16, kind="ExternalOutput")

    with TileContext(nc) as tc:
        with tc.tile_pool(name="persist_sb", bufs=1) as psb:
            # ---- persistent SBUF ----
            wo_sb = [psb.tile([128, _D], f16, name=f"wo{c}", tag=f"wo{c}")
                     for c in range(2)]
            qT = [psb.tile([128, _L], f16, name=f"qT{c}", tag=f"qT{c}")
                  for c in range(2)]
            kT = [psb.tile([128, _L], f16, name=f"kT{c}", tag=f"kT{c}")
                  for c in range(2)]
            # k blocks (0, NB-1) copied adjacent for the pair-0 stationary
            k031 = [psb.tile([128, 128], f16, name=f"k031{c}", tag=f"k031{c}")
                    for c in range(2)]
            # v' packed per head: 16 pairs x 65 cols; pair t rows 0:64 =
            # v[block a(t)], rows 64:128 = v[block b(t)], col 64 = ones
            vp = psb.tile([128, _HPC * 16 * 65], f16, name="vp", tag="vp")
            attnT = [psb.tile([128, _L], f16, name=f"attnT{c}",
                              tag=f"attnT{c}") for c in range(2)]
            for c in range(2):
                nc.sync.dma_start(wo_sb[c][:], wo[c * 128:(c + 1) * 128, :])
            # ones columns of v'
            for h in range(_HPC):
                nc.vector.memset(
                    vp[:, h * 1040 + 64: h * 1040 + 16 * 65: 65], 1.0)

            with tc.tile_pool(name="load_sb", bufs=1) as lsb:
                # ---- input DMA (w chunks first so PE can start early) ----
                xt_sb = [lsb.tile([128, _L], f16, name=f"xt{kc}",
                                  tag=f"xt{kc}") for kc in range(NKC)]
                wq_sb = [lsb.tile([128, 256], f16, name=f"wq{kc}",
                                  tag=f"wq{kc}") for kc in range(NKC)]
                wk_sb = [lsb.tile([128, 256], f16, name=f"wk{kc}",
                                  tag=f"wk{kc}") for kc in range(NKC)]
                wv_sb = [lsb.tile([128, 256], f16, name=f"wv{kc}",
                                  tag=f"wv{kc}") for kc in range(NKC)]
                for kc in range(NKC):
                    nc.sync.dma_start(wq_sb[kc][:], wq[kc * 128:(kc + 1) * 128, :])
                    nc.sync.dma_start(wk_sb[kc][:], wk[kc * 128:(kc + 1) * 128, :])
                    # xt chunk split in two so more DMA queues run in parallel
                    nc.sync.dma_start(xt_sb[kc][:, 0:1024],
                                      xt[kc * 128:(kc + 1) * 128, 0:1024])
                    nc.sync.dma_start(xt_sb[kc][:, 1024:2048],
                                      xt[kc * 128:(kc + 1) * 128, 1024:2048])
                for kc in range(NKC):
                    nc.sync.dma_start(wv_sb[kc][:], wv[kc * 128:(kc + 1) * 128, :])

                # ---- projections (Q, K interleaved per kc chunk) ----
                with tc.tile_pool(name="qk_ps", bufs=1, space="PSUM") as pps:
                    for half in range(2):  # L halves for earlier PE start
                        pt = {}
                        for wi in range(2):
                            for mc in range(2):
                                for nwi in range(2):
                                    pt[wi, mc, nwi] = pps.tile(
                                        [128, 512], f32,
                                        name=f"pp{wi}{mc}{nwi}",
                                        tag=f"pp{wi}{mc}{nwi}")
                        for kc in range(NKC):
                            for wi, w_sb in ((0, wq_sb), (1, wk_sb)):
                                for mc in range(2):
                                    for nwi in range(2):
                                        nw = half * 2 + nwi
                                        nc.tensor.matmul(
                                            pt[wi, mc, nwi][:],
                                            w_sb[kc][:, mc * 128:(mc + 1) * 128],
                                            xt_sb[kc][:, nw * 512:(nw + 1) * 512],
                                            start=(kc == 0),
                                            stop=(kc == NKC - 1))
                        for wi, dst in ((0, qT), (1, kT)):
                            for mc in range(2):
                                for nwi in range(2):
                                    nw = half * 2 + nwi
                                    nc.scalar.activation(
                                        dst[mc][:, nw * 512:(nw + 1) * 512],
                                        pt[wi, mc, nwi][:], COPY)
                # pair-0 stationary: k blocks 0 and NB-1 adjacent
                for c in range(2):
                    nc.vector.tensor_copy(k031[c][:, 0:64],
                                          kT[c][:, 0:64])
                    nc.vector.tensor_copy(k031[c][:, 64:128],
                                          kT[c][:, (_NB - 1) * 64:])
                # ---- V projection: shifted token tiles so that psum
                # partition rows land in score-pair layout ----
                with tc.tile_pool(name="v_ps", bufs=1, space="PSUM") as pps:
                    for t in range(16):
                        pv = pps.tile([128, 256], f32, name="pv", tag="pv",
                                      bufs=3)
                        if t == 0:
                            for kc in range(NKC):
                                nc.tensor.matmul(
                                    pv[0:64, :],
                                    xt_sb[kc][:, 0:64],
                                    wv_sb[kc][:],
                                    start=(kc == 0), stop=(kc == NKC - 1),
                                    tile_position=(0, 0))
                            for kc in range(NKC):
                                nc.tensor.matmul(
                                    pv[64:128, :],
                                    xt_sb[kc][:, (_NB - 1) * 64:],
                                    wv_sb[kc][:],
                                    start=(kc == 0), stop=(kc == NKC - 1),
                                    tile_position=(0, 64))
                        else:
                            for kc in range(NKC):
                                nc.tensor.matmul(
                                    pv[:],
                                    xt_sb[kc][:, (2 * t - 1) * 64:
                                              (2 * t + 1) * 64],
                                    wv_sb[kc][:],
                                    start=(kc == 0), stop=(kc == NKC - 1))
                        # scatter 4 heads into v' tile (pair index = t)
                        vdst = vp[:].rearrange("p (h c) -> p h c", c=1040)
                        vsrc = pv[:].rearrange("p (h d) -> p h d", d=64)
                        nc.vector.tensor_copy(
                            vdst[:, :, t * 65: t * 65 + 64], vsrc[:, :, :])

            with tc.tile_pool(name="att_sb", bufs=1) as asb:
                # ---- attention per head, key blocks in pairs ----
                with tc.tile_pool(name="att_ps", bufs=1, space="PSUM") as aps:

                    def head_ctx(h):
                        c, pb = h // 2, (h % 2) * 64
                        return dict(
                            h=h, c=c, pb=pb, hsec=h * 1040,
                            expS=asb.tile([128, nf * _FILLW], f16,
                                          name="expS", tag="expS", bufs=2),
                            outT=aps.tile([128, _L], f32, name="outT",
                                          tag="outT"),
                            oT_sb=asb.tile([65, _L], f16, name="oT_sb",
                                           tag="oT_sb", bufs=2),
                            rec=asb.tile([1, _L], f16, name="rec",
                                         tag="rec", bufs=2))

                    def emit_S(hc, fill, ps):
                        c, pb = hc["c"], hc["pb"]
                        for pc in fill:
                            t, q0, n, off = (pc["t"], pc["q0"], pc["n"],
                                             pc["off"])
                            if pc["g2"]:
                                rhs = qT[c][pb:pb + 64, :].rearrange(
                                    "p (a b) -> p a b",
                                    b=64)[:, 0:_NB:_NB - 1, :]
                            else:
                                rhs = qT[c][pb:pb + 64,
                                            q0 * 64:(q0 + n) * 64]
                            if t == 0:
                                lhsT = k031[c][pb:pb + 64, :]
                            else:
                                lhsT = kT[c][pb:pb + 64,
                                             (2 * t - 1) * 64:
                                             (2 * t + 1) * 64]
                            cols = 128 if pc["g2"] else n * 64
                            nc.tensor.matmul(
                                ps[0:128, off:off + cols], lhsT, rhs,
                                start=True, stop=True,
                                tile_position=(pb, 0))

                    def emit_exp_holes(hc, fi, fill, ps):
                        expS = hc["expS"]
                        nc.scalar.activation(
                            expS[:, fi * _FILLW:(fi + 1) * _FILLW],
                            ps[:], EXP)
                        nmem = 0
                        for pc in fill:
                            span = ([0, _NB - 1] if pc["g2"] else
                                    list(range(pc["q0"],
                                               pc["q0"] + pc["n"])))
                            for (half, qb) in pc["holes"]:
                                ci = span.index(qb)
                                hoff = fi * _FILLW + pc["off"] + ci * 64
                                eng = (nc.gpsimd if nmem % 2 == 0
                                       else nc.vector)
                                eng.memset(
                                    expS[half * 64:half * 64 + 64,
                                         hoff:hoff + 64], 0.0)
                                nmem += 1

                    def emit_AV(hc, fi, fill):
                        expS, outT, hsec = hc["expS"], hc["outT"], hc["hsec"]
                        for pi, pc in enumerate(fill):
                            t = pc["t"]
                            for ai, (x, nn, off2) in enumerate(pc["avs"]):
                                st, sp = plan["av_flags"][(fi, pi, ai)]
                                nc.tensor.matmul(
                                    outT[0:65, x * 64:(x + nn) * 64],
                                    vp[0:128, hsec + t * 65:
                                       hsec + t * 65 + 65],
                                    expS[0:128,
                                         fi * _FILLW + off2:
                                         fi * _FILLW + off2 + nn * 64],
                                    start=st, stop=sp,
                                    tile_position=(0, 0))

                    def emit_rec_dma(hc):
                        # evict outT (raw attn sums + denominator row), then
                        # reshape the sums row across 128 partitions via
                        # SBUF->SBUF DMA (single-partition reciprocal is slow)
                        nc.scalar.activation(hc["oT_sb"][0:65, :],
                                             hc["outT"][0:65, :], COPY)
                        recT = asb.tile([128, 16], f16, name="recT",
                                        tag="recT", bufs=2)
                        recT2 = asb.tile([128, 16], f16, name="recT2",
                                         tag="recT2", bufs=2)
                        nc.sync.dma_start(recT[:], hc["oT_sb"][64:65, :])
                        with nc.allow_low_precision("fp16 softmax denoms"):
                            nc.vector.reciprocal(recT2[:], recT[:])
                        nc.sync.dma_start(hc["rec"][:], recT2[:])

                    def emit_rec_pe(hc):
                        c, pb = hc["c"], hc["pb"]
                        # broadcast 1/sums across partitions on gpsimd (no
                        # PE or psum involvement), then all-SBUF f16 muls
                        recb = asb.tile([64, _L], f16, name="recb",
                                        tag="recb", bufs=2)
                        nc.gpsimd.partition_broadcast(recb[0:64, :],
                                                      hc["rec"][0:1, :])
                        for w in range(2):
                            nc.vector.tensor_mul(
                                attnT[c][pb:pb + 64,
                                         w * 1024:(w + 1) * 1024],
                                hc["oT_sb"][0:64, w * 1024:(w + 1) * 1024],
                                recb[0:64, w * 1024:(w + 1) * 1024])

                    # software pipeline across fills AND heads:
                    #   S(h,fi+1) | AV(h,fi) | exp(h,fi+1)
                    # with the last AV of head h deferred until after
                    # S(h+1,0), and head h's normalization injected after
                    # S(h+1,1).
                    fills = plan["fills"]
                    hcs = [head_ctx(h) for h in range(_HPC)]
                    prev = None  # (hc, fi) with AV not yet emitted
                    recq = []    # heads whose rec PE part is pending
                    for h in range(_HPC):
                        hc = hcs[h]
                        for fi, fill in enumerate(fills):
                            ps = aps.tile([128, _FILLW], f32, name="sfill",
                                          tag="sfill", bufs=2)
                            emit_S(hc, fill, ps)
                            if fi == 1 and recq:
                                emit_rec_pe(recq.pop(0))
                            if prev is not None:
                                emit_AV(*prev)
                            emit_exp_holes(hc, fi, fill, ps)
                            if fi == 0 and h > 0:
                                emit_rec_dma(hcs[h - 1])
                                recq.append(hcs[h - 1])
                            prev = (hc, fi, fill)
                    emit_AV(*prev)
                    emit_rec_dma(hcs[-1])
                    emit_rec_pe(hcs[-1])

                # ---- output projection ----
                with tc.tile_pool(name="o_ps", bufs=4, space="PSUM") as ops:
                    for m in range(NM):
                        po = [ops.tile([128, 512], f32, name="po",
                                       tag=f"po{nw}") for nw in range(2)]
                        for nw in range(2):
                            for c in range(2):
                                nc.tensor.matmul(
                                    po[nw][:],
                                    attnT[c][:, m * 128:(m + 1) * 128],
                                    wo_sb[c][:, nw * 512:(nw + 1) * 512],
                                    start=(c == 0), stop=(c == 1))
                        ob = asb.tile([128, _D], f16, name="ob", tag="ob",
                                      bufs=3)
                        for nw in range(2):
                            if nw == 0:
                                nc.scalar.activation(
                                    ob[:, nw * 512:(nw + 1) * 512],
                                    po[nw][:], COPY)
                            else:
                                nc.vector.tensor_copy(
                                    ob[:, nw * 512:(nw + 1) * 512],
                                    po[nw][:])
                            # half-tile DMAs engage more queues in parallel
                            nc.sync.dma_start(
                                out[m * 128:(m + 1) * 128,
                                    nw * 512:(nw + 1) * 512],
                                ob[:, nw * 512:(nw + 1) * 512])

    nc.finalize()
    return nc


def _get_plan_and_nc(sparse_mask):
    key = "nc"
    if key in _cache:
        return _cache[key]
    bm = np.asarray(sparse_mask)[::_BLK, ::_BLK]
    plan = _build_plan(bm)
    nc = _build_nc(plan)
    _cache[key] = (plan, nc)
    return plan, nc


def kernel(hidden_states, Wq, Wk, Wv, Wo, sparse_mask):
    from concourse.bass_utils import run_bass_kernel_spmd

    trace = bool(os.environ.get("BIGBIRD_TRACE"))
    if trace and "antenv.axon_hooks" not in sys.modules:
        try:
            import types

            import trn_agent_boot.trn_boot as _tb
            _hook = _tb._ntff_profile_via_ctypes("/opt/axon/libaxon_pjrt.so")
            _m = types.ModuleType("antenv.axon_hooks")
            _m.get_axon_ntff_profile_hook = lambda: _hook
            _m.set_axon_ntff_profile_hook = lambda h: None
            sys.modules["antenv.axon_hooks"] = _m
            import concourse.bass_utils as _bu
            _bu.upload_artifacts = lambda tmpdir: tmpdir
        except Exception as e:
            print(f"trace hook setup failed: {e}", file=sys.stderr)
            trace = False

    hs = np.asarray(hidden_states, np.float32)
    Wq = np.asarray(Wq, np.float32)
    Wk = np.asarray(Wk, np.float32)
    Wv = np.asarray(Wv, np.float32)
    Wo = np.asarray(Wo, np.float32)

    plan, nc = _get_plan_and_nc(sparse_mask)

    in_maps = []
    for core in range(_NCORES):
        b, hg = core // 4, core % 4
        hs_sl = slice(hg * 256, (hg + 1) * 256)
        in_maps.append({
            "xt": np.ascontiguousarray(hs[b].T).astype(np.float16),
            "wq": (np.ascontiguousarray(Wq[hs_sl].T)
                   * (1.0 / 8.0)).astype(np.float16),
            "wk": np.ascontiguousarray(Wk[hs_sl].T).astype(np.float16),
            "wv": np.ascontiguousarray(Wv[hs_sl].T).astype(np.float16),
            "wo": np.ascontiguousarray(Wo[:, hs_sl].T).astype(np.float16),
        })

    res = run_bass_kernel_spmd(nc, in_maps, list(range(_NCORES)), trace=trace)
    if trace:
        print(f"HW exec time: {res.exec_time_ns} ns")
        _cache["exec_time_ns"] = res.exec_time_ns

    out = np.zeros((_B, _L, _D), np.float32)
    for core in range(_NCORES):
        out[core // 4] += res.results[core]["out"].astype(np.float32)
    return out


# revision 13
# speedup vs baseline: 1.0898x; 1.0898x over previous
"""BigBird block-sparse attention on 8 Trainium2 NeuronCores.

Sharding: core = (batch b, head-group hg): b = core//4, hg = core%4.
Each core computes, for its batch and its 4 heads (all f16 inputs):
  qT/kT = (W{q,k}[hs] @ x.T)            [256, 2048]  (q pre-scaled by 1/8)
  v     = x @ Wv[hs].T                  [2048, 256]  in score-pair row layout
  Key blocks processed in PAIRS (0,31),(1,2),(3,4)..(29,30): one matmul
  computes transposed scores for both blocks of a pair (128 psum rows):
  S.T   = [k_a|k_b] @ q_span.T          write [128, span] per piece
  expS  = exp(S.T) packed in PSUM fills, evicted to SBUF (fp16),
          per-half holes zeroed (block kept by only one of a/b)
  outT  = [v_a|1 ; v_b|1].T @ expS      [65, 2048] accumulated in PSUM
  attnT = outT[0:64] * (1/outT[64]) per head  -> [256, 2048] f16
  out  += attnT.T @ Wo[:, hs].T         [2048, 1024] partial sum over heads
Host gathers: out[b] = sum over the 4 head-group cores of that batch.
"""

import os
import sys

import numpy as np

_B, _L, _D = 2, 2048, 1024
_H, _HD, _BLK = 16, 64, 64
_NB = _L // _BLK  # 32
_NCORES = 8
_HPC = 4  # heads per core
_FILLW = 1024  # packed-psum fill width (2 PSUM banks, f32)

_cache = {}


# --------------------------------------------------------------------------
# host-side plan: derive the block mask structure once
# --------------------------------------------------------------------------
def _build_plan(bm):
    """bm: [NB, NB] bool block mask (bm[i, j] = q-block i attends key-block j).

    Key blocks are processed in pairs: pair 0 = (0, NB-1), pair t>=1 =
    (2t-1, 2t).  Pair t's scores live on psum rows 0:64 (first block) and
    64:128 (second block) of shared q columns (the union of both blocks'
    kept q-blocks).

    Returns dict with:
      fills: list of fills; each is a list of pieces
             dict(t, q0, n, off, g2, avs, holes) where
             avs = [(a, nn, off2)] AV sub-pieces split at outT psum banks
             holes = [(half, q)] blocks to zero in expS after exp
      pair_blocks: [(a, b)] per pair t
      n_fills, av_flags
    """
    NB = bm.shape[0]
    GAP = int(os.environ.get("BIGBIRD_GAP", "0"))
    pairs = [(0, NB - 1)] + [(2 * t - 1, 2 * t) for t in range(1, NB // 2)]
    Q = [set(np.nonzero(bm[:, j])[0].tolist()) for j in range(NB)]

    fills = [[]]
    cur = [0]

    def close_fill():
        if fills[-1]:
            fills.append([])
        cur[0] = 0

    for t, (a, b) in enumerate(pairs):
        Qa, Qb = Q[a], Q[b]
        U = sorted(Qa | Qb)
        # merge q blocks into runs, allowing holes up to GAP blocks
        runs = []
        s = p = U[0]
        for x in U[1:]:
            if x - p <= GAP + 1:
                p = x
            else:
                runs.append((s, p))
                s = p = x
        runs.append((s, p))
        # fuse isolated {0} and {NB-1} singles into one strided matmul
        g2 = (len(runs) >= 2 and runs[0] == (0, 0)
              and runs[-1] == (NB - 1, NB - 1))
        if g2:
            runs = runs[1:-1]

        def emit(q0, n, is_g2):
            off = cur[0]
            cur[0] += (128 if is_g2 else n * _BLK)
            if is_g2:
                span = [0, NB - 1]
                avs = [(0, 1, off), (NB - 1, 1, off + 64)]
            else:
                span = list(range(q0, q0 + n))
                avs = []
                x = q0
                while x < q0 + n:
                    lim = min(q0 + n, ((x // 8) + 1) * 8)
                    avs.append((x, lim - x, off + (x - q0) * _BLK))
                    x = lim
            holes = ([(0, q) for q in span if q not in Qa]
                     + [(1, q) for q in span if q not in Qb])
            fills[-1].append(dict(t=t, q0=q0, n=n, off=off, g2=is_g2,
                                  avs=avs, holes=holes))

        if g2:
            # 128 cols; must not cross a 512-col psum bank boundary
            if cur[0] % 512 > 512 - 128:
                cur[0] = (cur[0] // 512 + 1) * 512
                if cur[0] >= _FILLW:
                    close_fill()
            emit(0, 2, True)
        for (s, e) in runs:
            q = s
            n_left = e - s + 1
            while n_left > 0:
                if cur[0] >= _FILLW:
                    close_fill()
                room = (512 - cur[0] % 512) // _BLK
                take = min(n_left, 8, room)
                emit(q, take, False)
                q += take
                n_left -= take
    if not fills[-1]:
        fills.pop()

    # outT psum-bank start/stop flags: first/last AV piece touching each
    # 512-col (8 q-block) bank, in emission order
    exec_order = []
    for fi, fill in enumerate(fills):
        for pi, pc in enumerate(fill):
            for ai, (x, nn, off2) in enumerate(pc["avs"]):
                exec_order.append(((fi, pi, ai), x // 8))
    first_seen = set()
    last_key = {}
    for key, bank in exec_order:
        last_key[bank] = key
    av_flags = {}
    for key, bank in exec_order:
        av_flags[key] = (bank not in first_seen, last_key[bank] == key)
        first_seen.add(bank)

    return dict(fills=fills, n_fills=len(fills), av_flags=av_flags,
                pair_blocks=pairs)


# --------------------------------------------------------------------------
# numpy simulator of the planned pipeline (used by test_plan.py)
# --------------------------------------------------------------------------
def _sim_plan(plan, q, k, v):
    """q, k, v: [L, 64] f32 (q pre-scaled by 1/8). Returns attn out [L, 64]."""
    nf = plan["n_fills"]
    pairs = plan["pair_blocks"]
    expS = np.zeros((128, nf * _FILLW), np.float32)
    outT = np.zeros((65, _L), np.float64)
    for fi, fill in enumerate(plan["fills"]):
        ps = np.zeros((128, _FILLW), np.float32)
        for pc in fill:
            t, q0, n, off = pc["t"], pc["q0"], pc["n"], pc["off"]
            a, b = pairs[t]
            if pc["g2"]:
                qsel = np.concatenate([q[0:64], q[(_NB - 1) * 64:]], axis=0)
            else:
                qsel = q[q0 * 64:(q0 + n) * 64]
            kp = np.concatenate([k[a * 64:(a + 1) * 64],
                                 k[b * 64:(b + 1) * 64]], axis=0)
            ps[:, off:off + qsel.shape[0]] = kp @ qsel.T
        expS[:, fi * _FILLW:(fi + 1) * _FILLW] = np.exp(ps)
        for pc in fill:
            span = ([0, _NB - 1] if pc["g2"]
                    else list(range(pc["q0"], pc["q0"] + pc["n"])))
            for (half, qb) in pc["holes"]:
                ci = span.index(qb)
                expS[half * 64:half * 64 + 64,
                     fi * _FILLW + pc["off"] + ci * 64:
                     fi * _FILLW + pc["off"] + (ci + 1) * 64] = 0.0
        for pc in fill:
            t = pc["t"]
            a, b = pairs[t]
            vp = np.concatenate([
                np.concatenate([v[a * 64:(a + 1) * 64],
                                np.ones((64, 1), np.float32)], axis=1),
                np.concatenate([v[b * 64:(b + 1) * 64],
                                np.ones((64, 1), np.float32)], axis=1),
            ], axis=0)  # [128, 65]
            for (x, nn, off2) in pc["avs"]:
                e = expS[:, fi * _FILLW + off2: fi * _FILLW + off2 + nn * 64]
                outT[:, x * 64:(x + nn) * 64] += vp.T @ e
    return (outT[0:64] / outT[64]).T


# --------------------------------------------------------------------------
# bass kernel build
# --------------------------------------------------------------------------
def _build_nc(plan):
    import concourse.bacc as bacc
    import concourse.mybir as mybir
    from concourse.tile import TileContext

    f32r = mybir.dt.float32r
    f32 = mybir.dt.float32
    f16 = mybir.dt.float16
    EXP = mybir.ActivationFunctionType.Exp
    COPY = mybir.ActivationFunctionType.Copy

    NKC = _D // 128   # 8 contraction chunks
    NM = _L // 128    # 16 L tiles
    nf = plan["n_fills"]
    pairs = plan["pair_blocks"]

    nc = bacc.Bacc(None, target_bir_lowering=False)

    xt = nc.dram_tensor("xt", [_D, _L], f16, kind="ExternalInput")
    wq = nc.dram_tensor("wq", [_D, 256], f16, kind="ExternalInput")
    wk = nc.dram_tensor("wk", [_D, 256], f16, kind="ExternalInput")
    wv = nc.dram_tensor("wv", [_D, 256], f16, kind="ExternalInput")
    wo = nc.dram_tensor("wo", [256, _D], f16, kind="ExternalInput")
    out = nc.dram_tensor("out", [_L, _D], f16, kind="ExternalOutput")

    with TileContext(nc) as tc:
        with tc.tile_pool(name="persist_sb", bufs=1) as psb:
            # ---- persistent SBUF ----
            wo_sb = [psb.tile([128, _D], f16, name=f"wo{c}", tag=f"wo{c}")
                     for c in range(2)]
            qT = [psb.tile([128, _L], f16, name=f"qT{c}", tag=f"qT{c}")
                  for c in range(2)]
            kT = [psb.tile([128, _L], f16, name=f"kT{c}", tag=f"kT{c}")
                  for c in range(2)]
            # k blocks (0, NB-1) copied adjacent for the pair-0 stationary
            k031 = [psb.tile([128, 128], f16, name=f"k031{c}", tag=f"k031{c}")
                    for c in range(2)]
            # v' packed per head: 16 pairs x 65 cols; pair t rows 0:64 =
            # v[block a(t)], rows 64:128 = v[block b(t)], col 64 = ones
            vp = psb.tile([128, _HPC * 16 * 65], f16, name="vp", tag="vp")
            attnT = [psb.tile([128, _L], f16, name=f"attnT{c}",
                              tag=f"attnT{c}") for c in range(2)]
            ones_sb = psb.tile([1, 64], f16, name="ones_sb", tag="ones_sb")
            nc.vector.memset(ones_sb[:], 1.0)
            for c in range(2):
                nc.sync.dma_start(wo_sb[c][:], wo[c * 128:(c + 1) * 128, :])
            # ones columns of v'
            for h in range(_HPC):
                nc.vector.memset(
                    vp[:, h * 1040 + 64: h * 1040 + 16 * 65: 65], 1.0)

            with tc.tile_pool(name="load_sb", bufs=1) as lsb:
                # ---- input DMA (w chunks first so PE can start early) ----
                xt_sb = [lsb.tile([128, _L], f16, name=f"xt{kc}",
                                  tag=f"xt{kc}") for kc in range(NKC)]
                wq_sb = [lsb.tile([128, 256], f16, name=f"wq{kc}",
                                  tag=f"wq{kc}") for kc in range(NKC)]
                wk_sb = [lsb.tile([128, 256], f16, name=f"wk{kc}",
                                  tag=f"wk{kc}") for kc in range(NKC)]
                wv_sb = [lsb.tile([128, 256], f16, name=f"wv{kc}",
                                  tag=f"wv{kc}") for kc in range(NKC)]
                for kc in range(NKC):
                    nc.sync.dma_start(wq_sb[kc][:], wq[kc * 128:(kc + 1) * 128, :])
                    nc.sync.dma_start(wk_sb[kc][:], wk[kc * 128:(kc + 1) * 128, :])
                    # xt chunk split in two so more DMA queues run in parallel
                    nc.sync.dma_start(xt_sb[kc][:, 0:1024],
                                      xt[kc * 128:(kc + 1) * 128, 0:1024])
                    nc.sync.dma_start(xt_sb[kc][:, 1024:2048],
                                      xt[kc * 128:(kc + 1) * 128, 1024:2048])
                for kc in range(NKC):
                    nc.sync.dma_start(wv_sb[kc][:], wv[kc * 128:(kc + 1) * 128, :])

                # ---- projections (Q, K interleaved per kc chunk) ----
                with tc.tile_pool(name="qk_ps", bufs=1, space="PSUM") as pps:
                    for half in range(2):  # L halves for earlier PE start
                        pt = {}
                        for wi in range(2):
                            for mc in range(2):
                                for nwi in range(2):
                                    pt[wi, mc, nwi] = pps.tile(
                                        [128, 512], f32,
                                        name=f"pp{wi}{mc}{nwi}",
                                        tag=f"pp{wi}{mc}{nwi}")
                        for kc in range(NKC):
                            for wi, w_sb in ((0, wq_sb), (1, wk_sb)):
                                for mc in range(2):
                                    for nwi in range(2):
                                        nw = half * 2 + nwi
                                        nc.tensor.matmul(
                                            pt[wi, mc, nwi][:],
                                            w_sb[kc][:, mc * 128:(mc + 1) * 128],
                                            xt_sb[kc][:, nw * 512:(nw + 1) * 512],
                                            start=(kc == 0),
                                            stop=(kc == NKC - 1))
                        for wi, dst in ((0, qT), (1, kT)):
                            for mc in range(2):
                                for nwi in range(2):
                                    nw = half * 2 + nwi
                                    nc.scalar.activation(
                                        dst[mc][:, nw * 512:(nw + 1) * 512],
                                        pt[wi, mc, nwi][:], COPY)
                # pair-0 stationary: k blocks 0 and NB-1 adjacent
                for c in range(2):
                    nc.vector.tensor_copy(k031[c][:, 0:64],
                                          kT[c][:, 0:64])
                    nc.vector.tensor_copy(k031[c][:, 64:128],
                                          kT[c][:, (_NB - 1) * 64:])
                # ---- V projection: shifted token tiles so that psum
                # partition rows land in score-pair layout ----
                with tc.tile_pool(name="v_ps", bufs=1, space="PSUM") as pps:
                    for t in range(16):
                        pv = pps.tile([128, 256], f32, name="pv", tag="pv",
                                      bufs=3)
                        if t == 0:
                            for kc in range(NKC):
                                nc.tensor.matmul(
                                    pv[0:64, :],
                                    xt_sb[kc][:, 0:64],
                                    wv_sb[kc][:],
                                    start=(kc == 0), stop=(kc == NKC - 1),
                                    tile_position=(0, 0))
                            for kc in range(NKC):
                                nc.tensor.matmul(
                                    pv[64:128, :],
                                    xt_sb[kc][:, (_NB - 1) * 64:],
                                    wv_sb[kc][:],
                                    start=(kc == 0), stop=(kc == NKC - 1),
                                    tile_position=(0, 64))
                        else:
                            for kc in range(NKC):
                                nc.tensor.matmul(
                                    pv[:],
                                    xt_sb[kc][:, (2 * t - 1) * 64:
                                              (2 * t + 1) * 64],
                                    wv_sb[kc][:],
                                    start=(kc == 0), stop=(kc == NKC - 1))
                        # scatter 4 heads into v' tile (pair index = t)
                        vdst = vp[:].rearrange("p (h c) -> p h c", c=1040)
                        vsrc = pv[:].rearrange("p (h d) -> p h d", d=64)
                        nc.vector.tensor_copy(
                            vdst[:, :, t * 65: t * 65 + 64], vsrc[:, :, :])

            with tc.tile_pool(name="att_sb", bufs=1) as asb:
                # ---- attention per head, key blocks in pairs ----
                with tc.tile_pool(name="att_ps", bufs=1, space="PSUM") as aps:

                    def head_ctx(h):
                        c, pb = h // 2, (h % 2) * 64
                        return dict(
                            h=h, c=c, pb=pb, hsec=h * 1040,
                            expS=asb.tile([128, nf * _FILLW], f16,
                                          name="expS", tag="expS", bufs=2),
                            outT=aps.tile([128, _L], f32, name="outT",
                                          tag="outT"),
                            oT_sb=asb.tile([65, _L], f16, name="oT_sb",
                                           tag="oT_sb", bufs=2),
                            rec=asb.tile([1, _L], f16, name="rec",
                                         tag="rec", bufs=2))

                    def emit_S(hc, fill, ps):
                        c, pb = hc["c"], hc["pb"]
                        for pc in fill:
                            t, q0, n, off = (pc["t"], pc["q0"], pc["n"],
                                             pc["off"])
                            if pc["g2"]:
                                rhs = qT[c][pb:pb + 64, :].rearrange(
                                    "p (a b) -> p a b",
                                    b=64)[:, 0:_NB:_NB - 1, :]
                            else:
                                rhs = qT[c][pb:pb + 64,
                                            q0 * 64:(q0 + n) * 64]
                            if t == 0:
                                lhsT = k031[c][pb:pb + 64, :]
                            else:
                                lhsT = kT[c][pb:pb + 64,
                                             (2 * t - 1) * 64:
                                             (2 * t + 1) * 64]
                            cols = 128 if pc["g2"] else n * 64
                            nc.tensor.matmul(
                                ps[0:128, off:off + cols], lhsT, rhs,
                                start=True, stop=True,
                                tile_position=(pb, 0))

                    def emit_exp_holes(hc, fi, fill, ps):
                        expS = hc["expS"]
                        nc.scalar.activation(
                            expS[:, fi * _FILLW:(fi + 1) * _FILLW],
                            ps[:], EXP)
                        nmem = 0
                        for pc in fill:
                            span = ([0, _NB - 1] if pc["g2"] else
                                    list(range(pc["q0"],
                                               pc["q0"] + pc["n"])))
                            for (half, qb) in pc["holes"]:
                                ci = span.index(qb)
                                hoff = fi * _FILLW + pc["off"] + ci * 64
                                eng = (nc.gpsimd if nmem % 2 == 0
                                       else nc.vector)
                                eng.memset(
                                    expS[half * 64:half * 64 + 64,
                                         hoff:hoff + 64], 0.0)
                                nmem += 1

                    def emit_AV(hc, fi, fill):
                        expS, outT, hsec = hc["expS"], hc["outT"], hc["hsec"]
                        for pi, pc in enumerate(fill):
                            t = pc["t"]
                            for ai, (x, nn, off2) in enumerate(pc["avs"]):
                                st, sp = plan["av_flags"][(fi, pi, ai)]
                                nc.tensor.matmul(
                                    outT[0:65, x * 64:(x + nn) * 64],
                                    vp[0:128, hsec + t * 65:
                                       hsec + t * 65 + 65],
                                    expS[0:128,
                                         fi * _FILLW + off2:
                                         fi * _FILLW + off2 + nn * 64],
                                    start=st, stop=sp,
                                    tile_position=(0, 0))

                    def emit_rec_dma(hc):
                        # evict outT (raw attn sums + denominator row), then
                        # reshape the sums row across 128 partitions via
                        # SBUF->SBUF DMA (single-partition reciprocal is slow)
                        nc.scalar.activation(hc["oT_sb"][0:65, :],
                                             hc["outT"][0:65, :], COPY)
                        recT = asb.tile([128, 16], f16, name="recT",
                                        tag="recT", bufs=2)
                        recT2 = asb.tile([128, 16], f16, name="recT2",
                                         tag="recT2", bufs=2)
                        nc.sync.dma_start(recT[:], hc["oT_sb"][64:65, :])
                        with nc.allow_low_precision("fp16 softmax denoms"):
                            nc.vector.reciprocal(recT2[:], recT[:])
                        nc.sync.dma_start(hc["rec"][:], recT2[:])

                    def emit_rec_pe(hc):
                        outT, c, pb = hc["outT"], hc["c"], hc["pb"]
                        # broadcast 1/sums into psum rows 64:128 via the PE
                        # (row 0:65 is free again: oT_sb eviction precedes)
                        for w in range(4):
                            nc.tensor.matmul(
                                outT[64:128, w * 512:(w + 1) * 512],
                                ones_sb[:],
                                hc["rec"][:, w * 512:(w + 1) * 512],
                                start=True, stop=True,
                                tile_position=(0, 64))
                        # single psum read to SBUF (partition-shifted copy is
                        # allowed; two-SBUF-input TT with mismatched base
                        # partitions is not), then an all-SBUF f16 mul
                        recb = asb.tile([64, _L], f16, name="recb",
                                        tag="recb", bufs=2)
                        nc.vector.tensor_copy(recb[0:64, :], outT[64:128, :])
                        for w in range(2):
                            nc.vector.tensor_mul(
                                attnT[c][pb:pb + 64,
                                         w * 1024:(w + 1) * 1024],
                                hc["oT_sb"][0:64, w * 1024:(w + 1) * 1024],
                                recb[0:64, w * 1024:(w + 1) * 1024])

                    # software pipeline across fills AND heads:
                    #   S(h,fi+1) | AV(h,fi) | exp(h,fi+1)
                    # with the last AV of head h deferred until after
                    # S(h+1,0), and head h's normalization injected after
                    # S(h+1,1).
                    fills = plan["fills"]
                    hcs = [head_ctx(h) for h in range(_HPC)]
                    prev = None  # (hc, fi) with AV not yet emitted
                    recq = []    # heads whose rec PE part is pending
                    for h in range(_HPC):
                        hc = hcs[h]
                        for fi, fill in enumerate(fills):
                            ps = aps.tile([128, _FILLW], f32, name="sfill",
                                          tag="sfill", bufs=2)
                            emit_S(hc, fill, ps)
                            if fi == 1 and recq:
                                emit_rec_pe(recq.pop(0))
                            if prev is not None:
                                emit_AV(*prev)
                            if fi == 0 and h > 0:
                                emit_rec_dma(hcs[h - 1])
                                recq.append(hcs[h - 1])
                            emit_exp_holes(hc, fi, fill, ps)
                            prev = (hc, fi, fill)
                    emit_AV(*prev)
                    emit_rec_dma(hcs[-1])
                    emit_rec_pe(hcs[-1])

                # ---- output projection ----
                with tc.tile_pool(name="o_ps", bufs=4, space="PSUM") as ops:
                    for m in range(NM):
                        po = [ops.tile([128, 512], f32, name="po",
                                       tag=f"po{nw}") for nw in range(2)]
                        for nw in range(2):
                            for c in range(2):
                                nc.tensor.matmul(
                                    po[nw][:],
                                    attnT[c][:, m * 128:(m + 1) * 128],
                                    wo_sb[c][:, nw * 512:(nw + 1) * 512],
                                    start=(c == 0), stop=(c == 1))
                        ob = asb.tile([128, _D], f16, name="ob", tag="ob",
                                      bufs=3)
                        for nw in range(2):
                            if nw == 0:
                                nc.scalar.activation(
                                    ob[:, nw * 512:(nw + 1) * 512],
                                    po[nw][:], COPY)
                            else:
                                nc.vector.tensor_copy(
                                    ob[:, nw * 512:(nw + 1) * 512],
                                    po[nw][:])
                            # half-tile DMAs engage more queues in parallel
                            nc.sync.dma_start(
                                out[m * 128:(m + 1) * 128,
                                    nw * 512:(nw + 1) * 512],
                                ob[:, nw * 512:(nw + 1) * 512])

    nc.finalize()
    return nc


def _get_plan_and_nc(sparse_mask):
    key = "nc"
    if key in _cache:
        return _cache[key]
    bm = np.asarray(sparse_mask)[::_BLK, ::_BLK]
    plan = _build_plan(bm)
    nc = _build_nc(plan)
    _cache[key] = (plan, nc)
    return plan, nc


def kernel(hidden_states, Wq, Wk, Wv, Wo, sparse_mask):
    from concourse.bass_utils import run_bass_kernel_spmd

    trace = bool(os.environ.get("BIGBIRD_TRACE"))
    if trace and "antenv.axon_hooks" not in sys.modules:
        try:
            import types

            import trn_agent_boot.trn_boot as _tb
            _hook = _tb._ntff_profile_via_ctypes("/opt/axon/libaxon_pjrt.so")
            _m = types.ModuleType("antenv.axon_hooks")
            _m.get_axon_ntff_profile_hook = lambda: _hook
            _m.set_axon_ntff_profile_hook = lambda h: None
            sys.modules["antenv.axon_hooks"] = _m
            import concourse.bass_utils as _bu
            _bu.upload_artifacts = lambda tmpdir: tmpdir
        except Exception as e:
            print(f"trace hook setup failed: {e}", file=sys.stderr)
            trace = False

    hs = np.asarray(hidden_states, np.float32)
    Wq = np.asarray(Wq, np.float32)
    Wk = np.asarray(Wk, np.float32)
    Wv = np.asarray(Wv, np.float32)
    Wo = np.asarray(Wo, np.float32)

    plan, nc = _get_plan_and_nc(sparse_mask)

    in_maps = []
    for core in range(_NCORES):
        b, hg = core // 4, core % 4
        hs_sl = slice(hg * 256, (hg + 1) * 256)
        in_maps.append({
            "xt": np.ascontiguousarray(hs[b].T).astype(np.float16),
            "wq": (np.ascontiguousarray(Wq[hs_sl].T)
                   * (1.0 / 8.0)).astype(np.float16),
            "wk": np.ascontiguousarray(Wk[hs_sl].T).astype(np.float16),
            "wv": np.ascontiguousarray(Wv[hs_sl].T).astype(np.float16),
            "wo": np.ascontiguousarray(Wo[:, hs_sl].T).astype(np.float16),
        })

    res = run_bass_kernel_spmd(nc, in_maps, list(range(_NCORES)), trace=trace)
    if trace:
        print(f"HW exec time: {res.exec_time_ns} ns")
        _cache["exec_time_ns"] = res.exec_time_ns

    out = np.zeros((_B, _L, _D), np.float32)
    for core in range(_NCORES):
        out[core // 4] += res.results[core]["out"].astype(np.float32)
    return out
